# revision 7
# baseline (speedup 1.0000x reference)
"""Trainium2 Bass kernel for nn_ModelNew_3556232921879.

Conv3d(3->16, k=3, VALID) + bias + channel multiplier + InstanceNorm3d +
clamp(-1,1) + channel multiplier + max over channels.

Math: with y = raw conv (no bias), m = multiplier[co], the bias and the sign
of m cancel under instance norm, so per (image, co):
    A = m^2 / sqrt(m^2*var(y) + eps),  B = -A*mean(y),  C = |m|
    out = max_co  median(A*y + B, -C, +C)
        = max_co  min(relu(A*y + (B + C)), 2C) - C

Per-core (data parallel over batch, 16 images/core):
  - conv as 12 matmuls/image: stationary Toeplitz-over-d weights
    S_kw[(kh,ci,d_in),(co*8+d')] (fp32r), moving = x rows loaded once per
    (image, d-block) as M2[(kh,ci,d_in), (h,w)] via one strided DMA.
  - PSUM quadrants [128=(co,d'), 512=(h',w)] for (db,hh); bn_stats/bn_aggr
    for per-partition stats; two tiny matmuls aggregate over d' partitions
    (G: [128,16] group-mean) and broadcast A,B back to 128 partitions (Rep).
  - ACT applies relu-affine straight out of PSUM; DVE tensor_scalar does
    min/shift; a 4-level partition tensor_tensor max tree reduces co;
    result [8=(d'),2048] DMAs out contiguously.
"""
import sys

if "/opt/trn_rl_repo" not in sys.path:
    sys.path.insert(0, "/opt/trn_rl_repo")

import numpy as np

import concourse.bacc as bacc
import concourse.bass as bass
import concourse.mybir as mybir
from concourse.tile import TileContext
from concourse.bass_utils import run_bass_kernel_spmd

EPS = 1e-5
N_CORES = 8
BS = 16  # images per core
F32 = mybir.dt.float32
F32R = mybir.dt.float32r
AF = mybir.ActivationFunctionType
OP = mybir.AluOpType

# x strides (elements) inside one image [3, 18, 34, 34]
X_CI, X_D, X_H = 18 * 1156, 1156, 34
IMG = 3 * X_CI          # 62424
OUT_IMG = 16 * 32 * 32  # 16384


def _build_stationaries(conv_w):
    # S_kw[p=(kh,ci,d_in), m=(co*8+d')] = w[co,ci,d_in-d',kh,kw] for 0<=d_in-d'<3
    S = np.zeros((3, 90, 128), np.float32)
    co = np.arange(16)
    for kw in range(3):
        for kh in range(3):
            for ci in range(3):
                for d_in in range(10):
                    p = kh * 30 + ci * 10 + d_in
                    for dp in range(max(0, d_in - 2), min(8, d_in + 1)):
                        S[kw, p, dp * 16 + co] = conv_w[:, ci, d_in - dp, kh, kw]
    return S


def _build_program(conv_w, multiplier):
    m = multiplier.reshape(16).astype(np.float32)
    S_np = _build_stationaries(conv_w)
    absm_np = np.tile(np.abs(m), 8).reshape(128, 1).astype(np.float32)
    m2co_np = (m * m).reshape(16, 1).astype(np.float32)
    G_np = np.zeros((128, 16), np.float32)       # group mean over d'
    for c in range(16):
        G_np[c::16, c] = 1.0 / 8.0
    Rep_np = np.zeros((16, 128), np.float32)     # broadcast co -> (d',co)
    for c in range(16):
        Rep_np[c, c::16] = 1.0

    nc = bacc.Bacc("TRN2", target_bir_lowering=False, debug=False,
                   num_devices=N_CORES)
    xs = nc.dram_tensor("xs", [BS, 3, 18, 34, 34], F32R, kind="ExternalInput")
    out = nc.dram_tensor("out", [BS, 16, 32, 32], F32, kind="ExternalOutput")

    c_S = [nc.inline_tensor(S_np[kw], f"S{kw}") for kw in range(3)]
    c_absm = nc.inline_tensor(absm_np, "absm")
    c_2absm = nc.inline_tensor(2.0 * absm_np, "absm2")
    c_negabsm = nc.inline_tensor(-absm_np, "negabsm")
    c_m2co = nc.inline_tensor(m2co_np, "m2co")
    c_G = nc.inline_tensor(G_np, "G")
    c_Rep = nc.inline_tensor(Rep_np, "Rep")

    with TileContext(nc) as tc:
        with (
            tc.tile_pool(name="consts", bufs=1) as consts,
            tc.tile_pool(name="m2", bufs=4) as m2p,
            tc.tile_pool(name="vbuf", bufs=2) as vbufp,
            tc.tile_pool(name="stats", bufs=2) as statp,
            tc.tile_pool(name="small", bufs=2) as smallp,
            tc.tile_pool(name="psum", bufs=6, space="PSUM") as psp,
            tc.tile_pool(name="psmall", bufs=1, space="PSUM") as pssm,
        ):
            # --- constants to SBUF (once) ---
            s_t = []
            for kw in range(3):
                sf = consts.tile([90, 128], F32, tag=f"sf{kw}")
                nc.gpsimd.dma_start(out=sf, in_=c_S[kw][:, :])
                sr = consts.tile([90, 128], F32R, tag=f"sr{kw}")
                nc.vector.tensor_copy(sr, sf)
                s_t.append(sr)
            t_absm = consts.tile([128, 1], F32, tag="absm")
            nc.gpsimd.dma_start(out=t_absm, in_=c_absm[:, :])
            t_2absm = consts.tile([128, 1], F32, tag="absm2")
            nc.gpsimd.dma_start(out=t_2absm, in_=c_2absm[:, :])
            t_negabsm = consts.tile([128, 1], F32, tag="negabsm")
            nc.gpsimd.dma_start(out=t_negabsm, in_=c_negabsm[:, :])
            t_m2co = consts.tile([16, 1], F32, tag="m2co")
            nc.gpsimd.dma_start(out=t_m2co, in_=c_m2co[:, :])
            t_G = consts.tile([128, 16], F32, tag="G")
            nc.gpsimd.dma_start(out=t_G, in_=c_G[:, :])
            t_Rep = consts.tile([16, 128], F32, tag="Rep")
            nc.gpsimd.dma_start(out=t_Rep, in_=c_Rep[:, :])

            for b in range(BS):
                ps_q = []
                st = statp.tile([128, 4, 6], F32, tag="st")
                for db in range(2):
                    # M2[(kh,ci,d_in), 1088] from x[b,:,db*8:db*8+10]; one
                    # DMA per kh (DMA APs are limited to 3 dims)
                    m2 = m2p.tile([90, 1088], F32R, tag=f"m2_{db}")
                    for kh in range(3):
                        src = bass.AP(
                            tensor=xs,
                            offset=b * IMG + db * 8 * X_D + kh * X_H,
                            ap=[[X_CI, 3], [X_D, 10], [1, 1088]],
                        )
                        nc.gpsimd.dma_start(
                            out=m2[kh * 30 : (kh + 1) * 30, :], in_=src)
                    m2v = m2[:, :].rearrange("p (h w) -> p h w", w=34)
                    for hh in range(2):
                        q = db * 2 + hh
                        ps = psp.tile([128, 512], F32, tag="conv")
                        for kw in range(3):
                            rhs = m2v[:, hh * 16 : hh * 16 + 16, kw : kw + 32]
                            nc.tensor.matmul(ps, s_t[kw], rhs,
                                             start=(kw == 0), stop=(kw == 2))
                        nc.vector.bn_stats(out=st[:, q, :], in_=ps[:, :])
                        ps_q.append(ps)

                # --- per-image stats -> A,B per partition ---
                mv = statp.tile([128, 2], F32, tag="mv")
                nc.vector.bn_aggr(out=mv, in_=st)
                musq = statp.tile([128, 1], F32, tag="musq")
                nc.vector.tensor_tensor(out=musq, in0=mv[:, 0:1], in1=mv[:, 0:1],
                                        op=OP.mult)
                nc.vector.tensor_tensor(out=mv[:, 1:2], in0=mv[:, 1:2],
                                        in1=musq, op=OP.add)
                g_ps = pssm.tile([16, 2], F32, tag="gps")
                nc.tensor.matmul(g_ps, t_G, mv[:, :], start=True, stop=True)
                g_sb = smallp.tile([16, 2], F32, tag="gsb")
                nc.vector.tensor_copy(g_sb, g_ps[:, :])
                # var_co = E2 - mu^2 ; den = m2*var + eps ; A = m2/sqrt(den)
                sm_musq = smallp.tile([16, 1], F32, tag="smusq")
                nc.vector.tensor_tensor(out=sm_musq, in0=g_sb[:, 0:1],
                                        in1=g_sb[:, 0:1], op=OP.mult)
                sm_var = smallp.tile([16, 1], F32, tag="svar")
                nc.vector.tensor_tensor(out=sm_var, in0=g_sb[:, 1:2],
                                        in1=sm_musq, op=OP.subtract)
                sm_den = smallp.tile([16, 1], F32, tag="sden")
                nc.vector.tensor_scalar(out=sm_den, in0=sm_var,
                                        scalar1=t_m2co[:, :], scalar2=EPS,
                                        op0=OP.mult, op1=OP.add)
                sm_sqrt = smallp.tile([16, 1], F32, tag="ssqrt")
                nc.scalar.activation(sm_sqrt, sm_den, AF.Sqrt)
                sm_rstd = smallp.tile([16, 1], F32, tag="srstd")
                nc.vector.reciprocal(sm_rstd, sm_sqrt)
                ab16 = smallp.tile([16, 2], F32, tag="ab16")
                nc.vector.tensor_tensor(out=ab16[:, 0:1], in0=sm_rstd,
                                        in1=t_m2co[:, :], op=OP.mult)
                sm_negmu = smallp.tile([16, 1], F32, tag="snegmu")
                nc.vector.tensor_scalar(out=sm_negmu, in0=g_sb[:, 0:1],
                                        scalar1=-1.0, scalar2=None, op0=OP.mult)
                nc.vector.tensor_tensor(out=ab16[:, 1:2], in0=ab16[:, 0:1],
                                        in1=sm_negmu, op=OP.mult)
                r_ps = pssm.tile([128, 2], F32, tag="rps")
                nc.tensor.matmul(r_ps, t_Rep, ab16[:, :], start=True, stop=True)
                ab128 = statp.tile([128, 2], F32, tag="ab128")
                nc.vector.tensor_copy(ab128, r_ps[:, :])
                bc = statp.tile([128, 1], F32, tag="bc")
                nc.vector.tensor_tensor(out=bc, in0=ab128[:, 1:2],
                                        in1=t_absm[:, :], op=OP.add)

                # --- relu-affine (ACT, frees PSUM), clip, max tree ---
                V = vbufp.tile([128, 2048], F32, tag="V")
                for q in range(4):
                    nc.scalar.activation(V[:, q * 512 : (q + 1) * 512],
                                         ps_q[q][:, :], AF.Relu,
                                         bias=bc[:, :], scale=ab128[:, 0:1])
                nc.gpsimd.tensor_scalar(out=V, in0=V, scalar1=t_2absm[:, :],
                                        scalar2=t_negabsm[:, :],
                                        op0=OP.min, op1=OP.add)
                # 32-block transpose: T[bp*32+w, (q*16+h')*32+(dlo*16+co)]
                # (V partitions are d'-major: p = d'*16+co, bp = d'//2,
                #  dlo = d'%2; V free f = q*512 + h'*32 + w)
                T = vbufp.tile([128, 2048], F32, tag="T")
                nc.vector.transpose(out=T, in_=V)
                # max over co (innermost, stride 1 after transpose)
                Tv = T[:, :].rearrange("p (bf dlo co) -> p dlo bf co",
                                       dlo=2, co=16)
                R = vbufp.tile([128, 128], F32, tag="R")
                nc.vector.tensor_reduce(out=R, in_=Tv,
                                        axis=mybir.AxisListType.X, op=OP.max)
                # R[bp*32+w, dlo*64+q*16+h'] ; fixup 32-block transpose:
                # RT[bp*32+hh*16+h', dlo*64+db*32+w]
                RT = vbufp.tile([128, 128], F32, tag="RT")
                nc.vector.transpose(out=RT, in_=R)
                # out[b, db*8+2*bp+dlo, hh*16+h', w]: one DMA per (db, dlo)
                for db in range(2):
                    for dlo in range(2):
                        dst = bass.AP(
                            tensor=out,
                            offset=b * OUT_IMG + db * 8192 + dlo * 1024,
                            ap=[[2048, 4], [1, 1024]],
                        )
                        nc.sync.dma_start(
                            out=dst,
                            in_=RT[:, dlo * 64 + db * 32 : dlo * 64 + db * 32 + 32])

    nc.finalize()
    return nc


_CACHE = {}


def kernel(x, conv_w, conv_b, multiplier):
    x = np.ascontiguousarray(x, np.float32)
    key = (conv_w.tobytes(), multiplier.tobytes())
    if key not in _CACHE:
        _CACHE[key] = _build_program(np.asarray(conv_w, np.float32),
                                     np.asarray(multiplier, np.float32))
    nc = _CACHE[key]
    in_maps = [{"xs": x[c * BS : (c + 1) * BS]} for c in range(N_CORES)]
    res = run_bass_kernel_spmd(nc, in_maps, list(range(N_CORES)))
    return np.concatenate([res.results[c]["out"] for c in range(N_CORES)], 0)
